# revision 27
# baseline (speedup 1.0000x reference)
"""Trainium2 Bass kernel for a causal single-head attention block.

Reference computation (fp32):
    q = x @ Wq; k = x @ Wk; v = x @ Wv        x: [B=256, T=256, C=384], W*: [384, 64]
    wei = softmax(causal_mask(q @ k.T / sqrt(C)))
    out = wei @ v                              out: [256, 256, 64]

Strategy: pure data parallel over B across 8 NeuronCores (32 batches/core).

All matmul operands are fp16 (full-rate PE streaming, fast weight load); PSUM
accumulation stays fp32. x is pre-transposed to [B, C, T] ON THE HOST so the
device never spends PE cycles transposing the activation, and the final
softmax division happens on the host too (the device ships raw o|Z fp32).

Per-batch dataflow (no PE transposes anywhere):
    xT      = plain DMA load (2 batches/DMA)  [128, 768] fp16 per batch
    qT|kT   = [Wq|Wk].T @ xT                  3 accumulating matmuls N=256, M=128
                                              (q on partitions 0-63, k on 64-127)
    kT      -> own base-0 tile                SBUF->SBUF DMA (cross-partition),
                                              issued 2 iterations before use and
                                              ahead of x loads in the sync ring
    v       = xT.T @ Wv                       6 matmuls N=64 (2 t-tiles x 3 chunks)
                                              into [v0|1|v1|1]; ones pre-written
    sT      = kT.T @ qT                       scores transposed [s, t]; causality
                                              skips the dead s1/t0 quadrant:
                                              N=256 (s-tile0) + N=128 (s-tile1)
    E       = exp(sT * 1/sqrt(C))             ONE fused ACT exp over [128, 384]
    M       = E * tri                         one strided GPSIMD multiply masks
                                              the two live triangles (s0/t0,
                                              s1/t1); s0/t1 needs no mask
    o|Z     = P.T @ [v | 1]                   P chunks stationary, v_ext moving:
                                              output lands NATURAL [t, 64+1],
                                              col 64 = softmax denominator Z
    store   = raw o|Z fp32                    one ACT copy [128, 130] per batch,
                                              one gpsimd DMA per 4 batches;
                                              host divides o by Z

Software pipeline per iteration (PE FIFO order):
    qkT(b+5) v(b+5) sc(b+3) o(b)
with softmax(b+3) emitted right after scores(b+3) and store(b-1) trailing by
one iteration. The sc->exp->mask chain spans three engines (PE->ACT->GPSIMD) and
~1.5-2us of FIFO+semaphore latency, so it gets THREE iterations of lead
before o(b+3) consumes it; store trails so its ACT copy never gates the next
exp in the ACT FIFO. Keeping the PE stream dense this way also keeps the HAM
clock-gate at 8/8 (2.4 GHz) -- micro-stalls were measured to re-throttle the
PE to 1.2 GHz mid-kernel, nearly doubling matmul durations.
"""

import numpy as np

N_EMBED = 384
HEAD_SIZE = 64
H1 = HEAD_SIZE + 1
T = 256
B = 256
N_CORES = 8
B_SHARD = B // N_CORES  # 32
CC = N_EMBED // 128  # 3 contraction chunks
INV_SQRT_C = 1.0 / float(np.sqrt(N_EMBED))

_CACHE = {}

# test.py can flip these before calling kernel()
TRACE = False
LAST_RESULTS = None


def _build_program():
    import concourse.bacc as bacc
    import concourse.mybir as mybir
    import concourse.tile as tile
    from concourse import bass

    f32 = mybir.dt.float32
    f16 = mybir.dt.float16
    ts = bass.ts
    Exp = mybir.ActivationFunctionType.Exp
    Copy = mybir.ActivationFunctionType.Copy

    nc = bacc.Bacc("TRN2", target_bir_lowering=False, debug=False,
                   enable_asserts=False)

    xt_d = nc.dram_tensor("xT", [B_SHARD, CC, 128, T], f16, kind="ExternalInput")
    wqk_d = nc.dram_tensor("Wqk", [CC, 128, 128], f16, kind="ExternalInput")
    wv_d = nc.dram_tensor("Wv", [CC, 128, HEAD_SIZE], f16, kind="ExternalInput")
    mask_d = nc.dram_tensor("mask01", [128, T], f16, kind="ExternalInput")
    # raw [o | Z] per t-tile; host divides
    out_d = nc.dram_tensor("oz", [B_SHARD, 2, 128, H1], f32, kind="ExternalOutput")

    xt_ap = xt_d.ap()
    out_ap = out_d.ap()

    with tile.TileContext(nc) as tc:
        with (
            tc.tile_pool(name="const", bufs=1) as cpool,
            tc.tile_pool(name="xin", bufs=7) as xin_pool,
            tc.tile_pool(name="proj", bufs=9) as proj_pool,
            tc.tile_pool(name="soft", bufs=7) as soft_pool,
            tc.tile_pool(name="outp", bufs=3) as out_pool,
            tc.tile_pool(name="ps_qk", bufs=2, space="PSUM") as ps_qk_pool,
            tc.tile_pool(name="ps_v", bufs=2, space="PSUM") as ps_v_pool,
            tc.tile_pool(name="ps_sc", bufs=2, space="PSUM") as ps_sc_pool,
            tc.tile_pool(name="ps_o", bufs=2, space="PSUM") as ps_o_pool,
        ):
            x_pairs = [None] * (B_SHARD // 2)

            def load_x2(j):
                # one DMA for batches 2j, 2j+1: [128, 1536] fp16 (~384 KB)
                t_ = xin_pool.tile([128, 2 * CC * T], f16, tag="xt2")
                nc.sync.dma_start(
                    t_[:].rearrange("p (bb cc t) -> p bb cc t", bb=2, cc=CC),
                    xt_ap[2 * j:2 * j + 2].rearrange("bb cc p t -> p bb cc t"))
                return t_

            # first x pair goes FIRST so proj(0) can start early; constants
            # ride the scalar HWDGE ring so they don't queue behind it
            x_pairs[0] = load_x2(0)
            wqk_sb, wv_sb = [], []
            for cc in range(CC):
                t_ = cpool.tile([128, 128], f16, tag=f"wqk{cc}")
                nc.scalar.dma_start(t_[:], wqk_d.ap()[cc])
                wqk_sb.append(t_)
                t_ = cpool.tile([128, HEAD_SIZE], f16, tag=f"wv{cc}")
                nc.scalar.dma_start(t_[:], wv_d.ap()[cc])
                wv_sb.append(t_)
            # tri mask duplicated: [tri | tri], tri[sl, tc] = (sl <= tc)
            mask_sb = cpool.tile([128, T], f16, tag="mask")
            nc.scalar.dma_start(mask_sb[:], mask_d.ap())
            # manual ring of [v0 | 1 | v1 | 1] tiles; ones written ONCE here
            v2_slots = []
            for i in range(8):
                t_ = cpool.tile([128, 2 * H1], f16, tag=f"v2_{i}")
                nc.gpsimd.memset(t_[:, HEAD_SIZE:H1], 1.0)
                nc.gpsimd.memset(t_[:, H1 + HEAD_SIZE:2 * H1], 1.0)
                v2_slots.append(t_)
            for j in range(1, 6):
                x_pairs[j] = load_x2(j)

            def proj(b):
                """qT|kT stacked [128, 256] (q rows 0-63, k rows 64-127) and
                v2 = [v0 | 1 | v1 | 1] natural [128, 130]."""
                xt2 = x_pairs[b // 2]
                off = (b % 2) * CC * T
                ps = ps_qk_pool.tile([128, 512], f32, tag="ps_qk")
                for cc in range(CC):
                    nc.tensor.matmul(ps[:, :T], wqk_sb[cc][:],
                                     xt2[:, off + cc * T: off + (cc + 1) * T],
                                     start=(cc == 0), stop=(cc == CC - 1))
                qk = proj_pool.tile([128, T], f16, tag="qk")
                nc.vector.tensor_copy(qk[:], ps[:, :T])
                # cross-partition move of kT to a base-0 tile
                kT = proj_pool.tile([HEAD_SIZE, T], f16, tag="kT")
                nc.sync.dma_start(kT[:], qk[HEAD_SIZE:128, :])

                psv = ps_v_pool.tile([128, 512], f32, tag="ps_v")
                for tt in range(2):
                    for cc in range(CC):
                        nc.tensor.matmul(
                            psv[:, tt * HEAD_SIZE:(tt + 1) * HEAD_SIZE],
                            xt2[:, off + cc * T + tt * 128:
                                off + cc * T + (tt + 1) * 128],
                            wv_sb[cc][:],
                            start=(cc == 0), stop=(cc == CC - 1))
                v2 = v2_slots[b % 8]
                # both t-tiles in one strided copy: psv cols {0:64, 64:128}
                # -> v2 cols {0:64, 65:129} (skipping the ones columns)
                nc.vector.tensor_copy(
                    v2[:].rearrange("p (g h) -> p g h", h=H1)[:, :, :HEAD_SIZE],
                    psv[:, :128].rearrange("p (g h) -> p g h", h=HEAD_SIZE))
                return qk, kT, v2

            def scores(qk, kT):
                """sT packed [128, 384]: cols 0-255 = s-tile0 (all t), cols
                256-383 = s-tile1 x t-tile1 (s1/t0 is fully causal-masked)."""
                ps = ps_sc_pool.tile([128, 512], f32, tag="ps_sc")
                nc.tensor.matmul(ps[:, :T], kT[:, :128], qk[:HEAD_SIZE, :],
                                 start=True, stop=True)
                nc.tensor.matmul(ps[:, T:T + 128], kT[:, 128:T],
                                 qk[:HEAD_SIZE, 128:T], start=True, stop=True)
                return ps

            def softmax(sc_ps):
                """E [128, 384] = exp (one fused ACT op); M [128, 256] = the
                two triangular blocks (cols 0:128 and 256:384 of E) masked by
                one strided DVE multiply. E[:, 128:256] (s0/t1) needs no mask."""
                e = soft_pool.tile([128, T + 128], f16, tag="e")
                nc.scalar.activation(e[:], sc_ps[:, :T + 128], Exp,
                                     scale=INV_SQRT_C)
                m = soft_pool.tile([128, T], f16, tag="m")
                # mask halves on different engines, matching the o-matmul's
                # dependency order (t0 <- DVE, t1a <- ACT exp, t1b <- gpsimd):
                # each consumer waits on a different engine, in arrival order
                nc.vector.tensor_mul(m[:, :128], e[:, :128], mask_sb[:, :128])
                nc.gpsimd.tensor_mul(m[:, 128:T], e[:, T:T + 128],
                                     mask_sb[:, :128])
                return e, m

            def o_mm(e, m, v2):
                """o|Z natural: [t-tile0 cols 0-64 | t-tile1 cols 65-129]."""
                ps = ps_o_pool.tile([128, 512], f32, tag="ps_o")
                nc.tensor.matmul(ps[:, :H1], m[:, :128], v2[:, :H1],
                                 start=True, stop=True)
                nc.tensor.matmul(ps[:, H1:2 * H1], e[:, 128:T], v2[:, :H1],
                                 start=True, stop=False)
                nc.tensor.matmul(ps[:, H1:2 * H1], m[:, 128:T], v2[:, H1:2 * H1],
                                 start=False, stop=True)
                return ps

            def store(b, ps, oquad):
                off = (b % 4) * 2 * H1
                nc.scalar.activation(oquad[:, off:off + 2 * H1], ps[:, :2 * H1],
                                     Copy)
                if b % 4 == 3:
                    nc.gpsimd.dma_start(
                        out_ap[b - 3:b + 1].rearrange("bb g p h -> p bb g h"),
                        oquad[:].rearrange("p (bb g h) -> p bb g h",
                                           bb=4, g=2))

            # ---- software-pipelined batch loop ----
            # PE stream per iteration: qkT(b+5) v(b+5) sc(b+3) o(b).
            # softmax(b+3) is emitted right after scores(b+3), THREE
            # iterations before o(b+3) consumes it: the exp->mask chain
            # (~1.5-2us with FIFO+sem latencies) must comfortably fit, else
            # the PE stalls at o's LDWEIGHTS, idles, and HAM re-throttles it
            # to 1.2 GHz. store(b-1) trails by one iteration so its ACT copy
            # never gates the next exp in the ACT FIFO.
            projs = {}
            for j in range(min(5, B_SHARD)):
                projs[j] = proj(j)
            soft = {}
            for j in range(min(3, B_SHARD)):
                soft[j] = softmax(scores(projs[j][0], projs[j][1]))
            oz_ps = {}
            oquads = {}
            for b in range(B_SHARD):
                if b % 4 == 0:
                    oq = out_pool.tile([128, 8 * H1], f32, tag="oq")
                    oquads[b // 4] = oq
                if b + 5 < B_SHARD:
                    projs[b + 5] = proj(b + 5)
                if b > 0:
                    store(b - 1, oz_ps.pop(b - 1), oquads[(b - 1) // 4])
                if b + 3 < B_SHARD:
                    nqk, nkT, _ = projs[b + 3]
                    soft[b + 3] = softmax(scores(nqk, nkT))
                e, m = soft.pop(b)
                _, _, v2 = projs.pop(b)
                oz_ps[b] = o_mm(e, m, v2)
                # x loads go LAST so kT moves never queue behind them
                if b % 2 == 0 and b // 2 + 6 < B_SHARD // 2:
                    x_pairs[b // 2 + 6] = load_x2(b // 2 + 6)
            store(B_SHARD - 1, oz_ps.pop(B_SHARD - 1),
                  oquads[(B_SHARD - 1) // 4])

    nc.compile()
    return nc


def _consts():
    # tri mask duplicated [128, 256] = [tri | tri]; tri[sl, tc] = (sl <= tc).
    # Block s-tile0/t-tile0 uses it with (s=sl, t=tc); block s-tile1/t-tile1
    # with (s=128+sl, t=128+tc) -- same triangle.
    sl = np.arange(128)[:, None]
    tri = (sl <= np.arange(128)[None, :])
    return np.concatenate([tri, tri], axis=1).astype(np.float16)


def _spot_check(out, x, Wq, Wk, Wv, batches):
    """Numpy reference for a few batches -- guards against transient device
    flakiness. The fp16 kernel's error is ~1e-3 abs; garbage is ~1e0."""
    for b in batches:
        xb = np.asarray(x[b], dtype=np.float32)
        q = xb @ Wq
        k = xb @ Wk
        v = xb @ Wv
        s = (q @ k.T) * np.float32(INV_SQRT_C)
        tmask = np.tril(np.ones((T, T), dtype=bool))
        s = np.where(tmask, s, -np.inf)
        w = np.exp(s - s.max(axis=-1, keepdims=True))
        o = (w @ v) / w.sum(axis=-1, keepdims=True)
        if np.max(np.abs(out[b] - o)) > 0.05 * max(np.max(np.abs(o)), 1e-3):
            return False
    return True


def kernel(x, Wq, Wk, Wv):
    global LAST_RESULTS
    from concourse import bass_utils

    if "nc" not in _CACHE:
        _CACHE["nc"] = _build_program()
    nc = _CACHE["nc"]

    # host-side prep: fp16 cast + transpose x to [B, C, T] (c-chunked view)
    x16t = np.ascontiguousarray(
        np.asarray(x, dtype=np.float16).transpose(0, 2, 1))
    x16t = x16t.reshape(B, CC, 128, T)
    # [Wq | Wk] stacked on the output dim, chunked along the contraction dim
    wqk = np.concatenate([np.asarray(Wq), np.asarray(Wk)], axis=1)
    wqk16 = np.ascontiguousarray(
        wqk.reshape(CC, 128, 2 * HEAD_SIZE), dtype=np.float16)
    wv16 = np.ascontiguousarray(
        np.asarray(Wv).reshape(CC, 128, HEAD_SIZE), dtype=np.float16)
    mask01 = _consts()

    in_maps = []
    for c in range(N_CORES):
        in_maps.append({
            "xT": x16t[c * B_SHARD:(c + 1) * B_SHARD],
            "Wqk": wqk16, "Wv": wv16, "mask01": mask01,
        })

    xf = np.ascontiguousarray(x, dtype=np.float32)
    Wqf = np.asarray(Wq, dtype=np.float32)
    Wkf = np.asarray(Wk, dtype=np.float32)
    Wvf = np.asarray(Wv, dtype=np.float32)
    check_batches = [c * B_SHARD for c in range(N_CORES)]
    for attempt in range(3):
        res = bass_utils.run_bass_kernel_spmd(
            nc, in_maps, core_ids=list(range(N_CORES)), trace=TRACE)
        LAST_RESULTS = res
        # oz: [B_SHARD, 2, 128, 65] per core -> host softmax division
        oz = np.concatenate([res.results[c]["oz"] for c in range(N_CORES)],
                            axis=0).astype(np.float32)
        out = np.ascontiguousarray(
            (oz[..., :HEAD_SIZE] / oz[..., HEAD_SIZE:]).reshape(
                B, T, HEAD_SIZE))
        if _spot_check(out, xf, Wqf, Wkf, Wvf, check_batches):
            return out
    return out
